# revision 36
# baseline (speedup 1.0000x reference)
"""Int8 LLaMA attention (torch-int Int8LlamaAttention) on 8 Trainium2 cores.

Sharding: TP=4 over heads x DP=2 over batch. Core c = 4*b + g handles
batch b, head-group g (8 heads, n-slice [1024g, 1024(g+1))).
Each core runs q/k/v projections for its n-slice, RoPE, int8 attention
(as bf16 matmuls - int8 values are exact in bf16, fp32 PSUM accumulation
is exact in range), and a partial o_proj over its k-slice. Host sums the
4 partials per batch (the "all-reduce") and applies output scale + bias.

All quantize steps use fused clip+round on DVE:
  pass1: u = min(x + M, M+127)   pass2: y = max(u - M, -128)
with M = 1.5*2^23 (magic-number round-to-nearest-even, exact for |x|<2^22).

Softmax (v2, single-pass normal layout):
  - QK^T in normal layout [s, t] per s-chunk -> row max (DVE reduce_max on
    PSUM) -> ACT exp(c*x - c*max) with per-partition bias, accum_out gives
    the row sum.
  - p~ = et * (127/sum): per-partition scalar multiply fused with the
    magic-round add (one DVE tensor_scalar), second tensor_scalar subtracts
    the magic -> integer-valued p in bf16, normal layout.
  - p transposed to [t, s] via DMA-transpose (idle DMA engines), feeding PV
    with no second QK pass, no PE transposes, no broadcast matmuls.
Heads are software-pipelined: projections of head h+1 overlap the softmax
chain of head h; PV(h) is emitted after QK(h+1) so the PE never waits on
the DVE/ACT/DMA chain.
"""

import math
import numpy as np
import ml_dtypes

import concourse.bass as bass
import concourse.tile as tile
import concourse.mybir as mybir
from concourse import bacc
from concourse.bass_utils import run_bass_kernel_spmd

# model dims
B, S, HID, NH, HD = 2, 1024, 4096, 32, 128
THETA = 10000.0
S_IN, S_W, S_B = 0.02, 0.01, 0.1
S_Q, S_K, S_V, S_O = 0.05, 0.05, 0.05, 0.05

NCORES = 8
TPG = 4            # tensor-parallel groups (head groups)
HPC = NH // TPG    # heads per core = 8
NSL = HPC * HD     # per-core n-slice width = 1024
SC = S             # per-core sequence (one batch per core) = 1024
KC = HID // 128    # k chunks = 32
NC_ = NSL // 128   # n chunks per core = 8 (== heads per core)
ST = SC // 512     # 512-wide s tiles = 2

ALPHA = float(np.float32(S_IN * S_W / S_Q))   # proj dequant scale (0.004)
CATT = float(np.float32(S_Q * S_K / math.sqrt(HD)))  # attn scale
C3 = float(np.float32((1.0 / 127.0) * S_V / S_O))    # pv dequant scale
MAGIC = 12582912.0   # 1.5 * 2^23

F32 = mybir.dt.float32
BF16 = mybir.dt.bfloat16
AX = mybir.AxisListType
OP = mybir.AluOpType
AF = mybir.ActivationFunctionType

_CACHE = {}


def _quant2(nc, out_ap, in_f32_ap):
    """clip(round(x)) to [-128, 127]: 2 fused DVE tensor_scalar ops."""
    nc.vector.tensor_scalar(in_f32_ap, in_f32_ap, MAGIC, MAGIC + 127.0,
                            OP.add, OP.min)
    nc.vector.tensor_scalar(out_ap, in_f32_ap, -MAGIC, -128.0,
                            OP.add, OP.max)


def build_nc(repeat=1):
    nc = bacc.Bacc("TRN2", target_bir_lowering=False, debug=False,
                   num_devices=NCORES)
    d = {}
    d["xt"] = nc.dram_tensor("xt", [KC, 128, SC], BF16, kind="ExternalInput")
    d["wq"] = nc.dram_tensor("wq", [NC_, 128, HID], BF16, kind="ExternalInput")
    d["wk"] = nc.dram_tensor("wk", [NC_, 128, HID], BF16, kind="ExternalInput")
    d["wv"] = nc.dram_tensor("wv", [NC_, 128, HID], BF16, kind="ExternalInput")
    d["wo"] = nc.dram_tensor("wo", [32, 128, NSL], BF16, kind="ExternalInput")
    d["cosT"] = nc.dram_tensor("cosT", [128, SC], F32, kind="ExternalInput")
    d["sinrT"] = nc.dram_tensor("sinrT", [128, SC], F32, kind="ExternalInput")
    d["bq"] = nc.dram_tensor("bq", [128, NC_], F32, kind="ExternalInput")
    d["bk"] = nc.dram_tensor("bk", [128, NC_], F32, kind="ExternalInput")
    d["bv"] = nc.dram_tensor("bv", [128, NC_], F32, kind="ExternalInput")
    d["ot"] = nc.dram_tensor("ot", [HID, SC], F32, kind="ExternalOutput")

    with tile.TileContext(nc) as tc:
        for _ in range(repeat):
            _emit(nc, tc, d)
    nc.compile()
    return nc


def _emit(nc, tc, d):
    import contextlib
    es = contextlib.ExitStack()
    with es:
        const = es.enter_context(tc.tile_pool(name="const", bufs=1))
        persist = es.enter_context(tc.tile_pool(name="persist", bufs=1))

        # ctx^T [d(+128h), s] accumulated across heads, consumed by o_proj
        ctxT = persist.tile([128, NC_, SC], BF16)

        with tc.tile_pool(name="xt", bufs=1) as xtp, \
             tc.tile_pool(name="wproj", bufs=3) as wp, \
             tc.tile_pool(name="pps", bufs=2, space="PSUM") as pps, \
             tc.tile_pool(name="aps", bufs=2, space="PSUM") as aps, \
             tc.tile_pool(name="cps", bufs=2, space="PSUM") as cps, \
             tc.tile_pool(name="pscr", bufs=2) as pscr, \
             tc.tile_pool(name="qk", bufs=2) as qkp, \
             tc.tile_pool(name="vna", bufs=3) as vnp, \
             tc.tile_pool(name="stat", bufs=16) as stat, \
             tc.tile_pool(name="et", bufs=2) as etp, \
             tc.tile_pool(name="pb", bufs=4) as pbp, \
             tc.tile_pool(name="ptr", bufs=2) as ptrp:

            # per-chunk tiles so each matmul gates on its own chunk's DMA;
            # x rides the Sync HWDGE ring, weights ride the Scalar ring.
            # rope tables + biases (first used ~35us in) go behind the first
            # few x chunks so they don't delay the first matmul.
            xts = [xtp.tile([128, SC], BF16, tag=f"x{kc}", name=f"x{kc}")
                   for kc in range(KC)]
            for kc in range(4):
                nc.sync.dma_start(xts[kc][:], d["xt"][kc, :, :])
            cosT = const.tile([128, SC], F32)
            nc.sync.dma_start(cosT[:], d["cosT"][:, :])
            sinrT = const.tile([128, SC], F32)
            nc.sync.dma_start(sinrT[:], d["sinrT"][:, :])
            bq = const.tile([128, NC_], F32)
            nc.sync.dma_start(bq[:], d["bq"][:, :])
            bk = const.tile([128, NC_], F32)
            nc.sync.dma_start(bk[:], d["bk"][:, :])
            bv = const.tile([128, NC_], F32)
            nc.sync.dma_start(bv[:], d["bv"][:, :])
            for kc in range(4, KC):
                nc.sync.dma_start(xts[kc][:], d["xt"][kc, :, :])

            def proj_nc(wdram, bias_t, ncx, out_cb, split_w=1):
                """y[n 128, s 1024] = dequant(w_chunk @ x) for n-chunk ncx."""
                wt = wp.tile([128, KC, 128], BF16, tag="w")
                kq = KC // split_w
                for j in range(split_w):
                    nc.scalar.dma_start(wt[:, j * kq:(j + 1) * kq, :],
                                        wdram[ncx, :, j * kq * 128:
                                              (j + 1) * kq * 128])
                y = pscr.tile([128, SC], F32, tag="y")
                for half in range(ST):
                    fsl = bass.ts(half, 512)
                    ps = pps.tile([128, 512], F32, tag="ps")
                    for kc in range(KC):
                        nc.tensor.matmul(ps[:], wt[:, kc, :],
                                         xts[kc][:, fsl],
                                         start=(kc == 0), stop=(kc == KC - 1))
                    nc.scalar.activation(y[:, fsl], ps[:], AF.Identity,
                                         bias=bias_t[:, ncx:ncx + 1],
                                         scale=ALPHA)
                out_cb(ncx, y)

            def q_like(dst, ncx, y):
                """quant -> RoPE (fp32) -> quant -> bf16 [d 128, s 1024].

                y is clobbered (quant pass1 is in-place) and then reused as
                the rotate-half product buffer; qi is rotated in place."""
                qi = pscr.tile([128, SC], F32, tag="qi")
                _quant2(nc, qi[:], y[:])
                nc.vector.tensor_mul(y[0:64, :], qi[64:128, :],
                                     sinrT[64:128, :])
                nc.vector.tensor_mul(y[64:128, :], qi[0:64, :],
                                     sinrT[0:64, :])
                nc.vector.tensor_mul(qi[:], qi[:], cosT[:])
                nc.vector.tensor_add(qi[:], qi[:], y[:])
                _quant2(nc, dst[:], qi[:])

            def v_like(dst_vn, ncx, y):
                vt = pscr.tile([128, SC], BF16, tag="vt")
                _quant2(nc, vt[:], y[:])
                # v^T [d, t] -> v natural [t, d] via DMA transpose (2-byte)
                nc.sync.dma_start_transpose(dst_vn[:], vt[:])

            def qk_head(qTh, kTh, ptr):
                """Single-pass softmax, normal layout; fills ptr [t,s] bf16."""
                for sc in range(NC_):
                    pa = aps.tile([128, SC], F32, tag="attn")
                    for half in range(ST):
                        fsl = bass.ts(half, 512)
                        nc.tensor.matmul(pa[:, fsl], qTh[:, bass.ts(sc, 128)],
                                         kTh[:, fsl], start=True, stop=True)
                    mx = stat.tile([128, 1], F32, tag="mx")
                    nc.vector.reduce_max(mx[:], pa[:], axis=AX.X)
                    nb = stat.tile([128, 1], F32, tag="nb")
                    nc.vector.tensor_scalar_mul(nb[:], mx[:], -CATT)
                    sm = stat.tile([128, 1], F32, tag="sm")
                    et = etp.tile([128, SC], F32, tag="et")
                    nc.scalar.activation(et[:], pa[:], AF.Exp,
                                         bias=nb[:], scale=CATT,
                                         accum_out=sm[:])
                    smk = stat.tile([128, 1], F32, tag="smk")
                    nc.vector.tensor_scalar_mul(smk[:], sm[:], 1.0 / 127.0)
                    rs = stat.tile([128, 1], F32, tag="rs")
                    nc.vector.reciprocal(rs[:], smk[:])
                    # p~ = et * (127/sum), magic-rounded to ints, bf16
                    # (first pass in-place on et)
                    nc.vector.tensor_scalar(et[:], et[:], rs[:], MAGIC,
                                            OP.mult, OP.add)
                    pb = pbp.tile([128, SC], BF16, tag="pb")
                    last = nc.gpsimd.tensor_scalar_add(pb[:], et[:], -MAGIC)
                    # [s-chunk 128, t 1024] -> ptr[t%128, sc, t//128, 128]
                    nc.sync.dma_start_transpose(ptr[:, sc, :, :], pb[:])
                return last

            def pv_head(h, vn, ptr):
                for half in range(ST):
                    fsl = bass.ts(half, 512)
                    pc = cps.tile([128, 512], F32, tag="ctx")
                    for c in range(NC_):
                        nc.tensor.matmul(pc[:], vn[:, c, :],
                                         ptr[:, half * 4:half * 4 + 4, c, :],
                                         start=(c == 0), stop=(c == NC_ - 1))
                    cf = pscr.tile([128, 512], F32, tag="cf")
                    nc.scalar.activation(cf[:], pc[:], AF.Copy, scale=C3)
                    _quant2(nc, ctxT[:, h, fsl], cf[:])

            # ---- head-pipelined projections + attention ----
            # pv lags one head behind qk so the PE has projection work to
            # chew on while each head's softmax chain completes.
            prev = None
            for h in range(HPC):
                qTh = qkp.tile([128, SC], BF16, tag="qT")
                kTh = qkp.tile([128, SC], BF16, tag="kT")
                vn = vnp.tile([128, NC_, 128], BF16, tag="vn")
                ptr = ptrp.tile([128, NC_, NC_, 128], BF16, tag="ptr")
                sw = 4 if h == 0 else 1
                proj_nc(d["wq"], bq, h, lambda i, y: q_like(qTh, i, y), sw)
                proj_nc(d["wk"], bk, h, lambda i, y: q_like(kTh, i, y), sw)
                proj_nc(d["wv"], bv, h, lambda i, y: v_like(vn, i, y), sw)
                qk_head(qTh, kTh, ptr)
                if prev is not None:
                    pv_head(*prev)
                prev = (h, vn, ptr)
            pv_head(*prev)

        # ---------------- o_proj partial ----------------
        with tc.tile_pool(name="wo", bufs=3) as wop, \
             tc.tile_pool(name="ops", bufs=4, space="PSUM") as ops, \
             tc.tile_pool(name="oscr", bufs=3) as oscr:
            for mc in range(32):
                wt = wop.tile([128, NC_, 128], BF16, tag="wo")
                nc.scalar.dma_start(wt[:], d["wo"][mc, :, :])
                for half in range(ST):
                    fsl = bass.ts(half, 512)
                    po = ops.tile([128, 512], F32, tag="po")
                    for kcx in range(NC_):
                        nc.tensor.matmul(po[:], wt[:, kcx, :],
                                         ctxT[:, kcx, fsl],
                                         start=(kcx == 0), stop=(kcx == NC_ - 1))
                    ob = oscr.tile([128, 512], F32, tag="ob")
                    nc.scalar.copy(ob[:], po[:])
                    nc.sync.dma_start(d["ot"][bass.ts(mc, 128), fsl], ob[:])


# ---------------- host side ----------------

def _rope_tables_np(pos_row):
    j = np.arange(0, HD, 2, dtype=np.float32) / np.float32(HD)
    inv = np.float32(1.0) / np.power(np.float32(THETA), j)
    freqs = pos_row.astype(np.float32)[:, None] * inv[None, :]   # [S, 64]
    emb = np.concatenate([freqs, freqs], axis=-1)                # [S, 128]
    cosT = np.ascontiguousarray(np.cos(emb).T.astype(np.float32))
    sinT = np.sin(emb).T.astype(np.float32)
    sinr = sinT.copy()
    sinr[0:HD // 2] *= np.float32(-1.0)
    # rotate by 64 partitions so rope muls have base-aligned inputs:
    # sins[d] = sinrot[(d+64) % 128]
    sins = np.concatenate([sinr[HD // 2:], sinr[:HD // 2]], axis=0)
    return cosT, np.ascontiguousarray(sins)


def _prep_inputs(hidden_states, position_ids, w_q, w_k, w_v, w_o,
                 b_q, b_k, b_v):
    bf = ml_dtypes.bfloat16
    in_maps = []
    x = np.asarray(hidden_states, dtype=np.float32)
    x_i8 = np.clip(np.round(x / np.float32(S_IN)), -128, 127)
    for c in range(NCORES):
        b, g = c // TPG, c % TPG
        gsl = slice(g * NSL, (g + 1) * NSL)
        xt = np.ascontiguousarray(x_i8[b].T).reshape(KC, 128, SC).astype(bf)
        def wslice(w):
            wg = np.asarray(w[gsl], dtype=np.float32)     # [1024, 4096]
            t = wg.reshape(NC_, 128, KC, 128).transpose(0, 3, 2, 1)
            return np.ascontiguousarray(t.reshape(NC_, 128, HID)).astype(bf)
        wog = np.asarray(w_o[:, gsl], dtype=np.float32)   # [4096, 1024]
        wo = wog.reshape(32, 128, NC_, 128).transpose(0, 3, 2, 1)
        wo = np.ascontiguousarray(wo.reshape(32, 128, NSL)).astype(bf)
        cosT, sinrT = _rope_tables_np(np.asarray(position_ids)[b])
        bs = lambda bb, s: np.ascontiguousarray(
            (np.asarray(bb[gsl], dtype=np.float32) * np.float32(s))
            .reshape(NC_, 128).T)
        in_maps.append({
            "xt": xt, "wq": wslice(w_q), "wk": wslice(w_k), "wv": wslice(w_v),
            "wo": wo, "cosT": cosT, "sinrT": sinrT,
            "bq": bs(b_q, S_B / S_Q), "bk": bs(b_k, S_B / S_K),
            "bv": bs(b_v, S_B / S_V),
        })
    return in_maps


def _finish(results, b_o):
    out = np.empty((B, S, HID), dtype=np.float32)
    sc = np.float32(S_O * S_W)
    bo = np.asarray(b_o, dtype=np.float32)
    for b in range(B):
        acc = np.zeros((HID, SC), dtype=np.float32)
        for g in range(TPG):
            acc += results[b * TPG + g]["ot"]
        out[b] = acc.T * sc + bo[None, :]
    return out


def kernel(hidden_states, position_ids, w_q, w_k, w_v, w_o,
           b_q, b_k, b_v, b_o):
    if "nc" not in _CACHE:
        _CACHE["nc"] = build_nc()
    nc = _CACHE["nc"]
    in_maps = _prep_inputs(hidden_states, position_ids, w_q, w_k, w_v, w_o,
                           b_q, b_k, b_v)
    res = run_bass_kernel_spmd(nc, in_maps, core_ids=list(range(NCORES)))
    return _finish(res.results, b_o)


# revision 37
# speedup vs baseline: 2.4056x; 2.4056x over previous
"""Int8 LLaMA attention (torch-int Int8LlamaAttention) on 8 Trainium2 cores.

Sharding: TP=4 over heads x DP=2 over batch. Core c = 4*b + g handles
batch b, head-group g (8 heads, n-slice [1024g, 1024(g+1))).
Each core runs q/k/v projections for its n-slice, RoPE, int8 attention
(as bf16 matmuls - int8 values are exact in bf16, fp32 PSUM accumulation
is exact in range), and a partial o_proj over its k-slice. Host sums the
4 partials per batch (the "all-reduce") and applies output scale + bias.

All quantize steps use fused clip+round on DVE:
  pass1: u = min(x + M, M+127)   pass2: y = max(u - M, -128)
with M = 1.5*2^23 (magic-number round-to-nearest-even, exact for |x|<2^22).

Softmax (v2, single-pass normal layout):
  - QK^T in normal layout [s, t] per s-chunk -> row max (DVE reduce_max on
    PSUM) -> ACT exp(c*x - c*max) with per-partition bias, accum_out gives
    the row sum.
  - p~ = et * (127/sum): per-partition scalar multiply fused with the
    magic-round add (one DVE tensor_scalar), second tensor_scalar subtracts
    the magic -> integer-valued p in bf16, normal layout.
  - p transposed to [t, s] via DMA-transpose (idle DMA engines), feeding PV
    with no second QK pass, no PE transposes, no broadcast matmuls.
Heads are software-pipelined: projections of head h+1 overlap the softmax
chain of head h; PV(h) is emitted after QK(h+1) so the PE never waits on
the DVE/ACT/DMA chain.
"""

import math
import numpy as np
import ml_dtypes

import concourse.bass as bass
import concourse.tile as tile
import concourse.mybir as mybir
from concourse import bacc
from concourse.bass_utils import run_bass_kernel_spmd

# model dims
B, S, HID, NH, HD = 2, 1024, 4096, 32, 128
THETA = 10000.0
S_IN, S_W, S_B = 0.02, 0.01, 0.1
S_Q, S_K, S_V, S_O = 0.05, 0.05, 0.05, 0.05

NCORES = 8
TPG = 4            # tensor-parallel groups (head groups)
HPC = NH // TPG    # heads per core = 8
NSL = HPC * HD     # per-core n-slice width = 1024
SC = S             # per-core sequence (one batch per core) = 1024
KC = HID // 128    # k chunks = 32
NC_ = NSL // 128   # n chunks per core = 8 (== heads per core)
ST = SC // 512     # 512-wide s tiles = 2

ALPHA = float(np.float32(S_IN * S_W / S_Q))   # proj dequant scale (0.004)
CATT = float(np.float32(S_Q * S_K / math.sqrt(HD)))  # attn scale
C3 = float(np.float32((1.0 / 127.0) * S_V / S_O))    # pv dequant scale
MAGIC = 12582912.0   # 1.5 * 2^23

F32 = mybir.dt.float32
BF16 = mybir.dt.bfloat16
AX = mybir.AxisListType
OP = mybir.AluOpType
AF = mybir.ActivationFunctionType

_CACHE = {}


def _quant2(nc, out_ap, in_f32_ap):
    """clip(round(x)) to [-128, 127]: 2 fused DVE tensor_scalar ops."""
    nc.vector.tensor_scalar(in_f32_ap, in_f32_ap, MAGIC, MAGIC + 127.0,
                            OP.add, OP.min)
    nc.vector.tensor_scalar(out_ap, in_f32_ap, -MAGIC, -128.0,
                            OP.add, OP.max)


def build_nc(repeat=1):
    nc = bacc.Bacc("TRN2", target_bir_lowering=False, debug=False,
                   num_devices=NCORES)
    d = {}
    d["xt"] = nc.dram_tensor("xt", [KC, 128, SC], BF16, kind="ExternalInput")
    d["wq"] = nc.dram_tensor("wq", [NC_, 128, HID], BF16, kind="ExternalInput")
    d["wk"] = nc.dram_tensor("wk", [NC_, 128, HID], BF16, kind="ExternalInput")
    d["wv"] = nc.dram_tensor("wv", [NC_, 128, HID], BF16, kind="ExternalInput")
    d["wo"] = nc.dram_tensor("wo", [32, 128, NSL], BF16, kind="ExternalInput")
    d["cosT"] = nc.dram_tensor("cosT", [128, SC], F32, kind="ExternalInput")
    d["sinrT"] = nc.dram_tensor("sinrT", [128, SC], F32, kind="ExternalInput")
    d["bq"] = nc.dram_tensor("bq", [128, NC_], F32, kind="ExternalInput")
    d["bk"] = nc.dram_tensor("bk", [128, NC_], F32, kind="ExternalInput")
    d["bv"] = nc.dram_tensor("bv", [128, NC_], F32, kind="ExternalInput")
    d["ot"] = nc.dram_tensor("ot", [HID, SC], F32, kind="ExternalOutput")

    with tile.TileContext(nc) as tc:
        for _ in range(repeat):
            _emit(nc, tc, d)
    nc.compile()
    return nc


def _emit(nc, tc, d):
    import contextlib
    es = contextlib.ExitStack()
    with es:
        const = es.enter_context(tc.tile_pool(name="const", bufs=1))
        persist = es.enter_context(tc.tile_pool(name="persist", bufs=1))

        # ctx^T [d(+128h), s] accumulated across heads, consumed by o_proj
        ctxT = persist.tile([128, NC_, SC], BF16)

        with tc.tile_pool(name="xt", bufs=1) as xtp, \
             tc.tile_pool(name="wproj", bufs=3) as wp, \
             tc.tile_pool(name="pps", bufs=2, space="PSUM") as pps, \
             tc.tile_pool(name="aps", bufs=2, space="PSUM") as aps, \
             tc.tile_pool(name="cps", bufs=2, space="PSUM") as cps, \
             tc.tile_pool(name="pscr", bufs=2) as pscr, \
             tc.tile_pool(name="qk", bufs=2) as qkp, \
             tc.tile_pool(name="vna", bufs=3) as vnp, \
             tc.tile_pool(name="stat", bufs=16) as stat, \
             tc.tile_pool(name="et", bufs=2) as etp, \
             tc.tile_pool(name="pb", bufs=4) as pbp, \
             tc.tile_pool(name="ptr", bufs=2) as ptrp:

            # per-chunk tiles so each matmul gates on its own chunk's DMA;
            # x rides the Sync HWDGE ring, weights ride the Scalar ring.
            # rope tables + biases (first used ~35us in) go behind the first
            # few x chunks so they don't delay the first matmul.
            xts = [xtp.tile([128, SC], BF16, tag=f"x{kc}", name=f"x{kc}")
                   for kc in range(KC)]
            for kc in range(4):
                nc.sync.dma_start(xts[kc][:], d["xt"][kc, :, :])
            cosT = const.tile([128, SC], F32)
            nc.sync.dma_start(cosT[:], d["cosT"][:, :])
            sinrT = const.tile([128, SC], F32)
            nc.sync.dma_start(sinrT[:], d["sinrT"][:, :])
            bq = const.tile([128, NC_], F32)
            nc.sync.dma_start(bq[:], d["bq"][:, :])
            bk = const.tile([128, NC_], F32)
            nc.sync.dma_start(bk[:], d["bk"][:, :])
            bv = const.tile([128, NC_], F32)
            nc.sync.dma_start(bv[:], d["bv"][:, :])
            for kc in range(4, KC):
                nc.sync.dma_start(xts[kc][:], d["xt"][kc, :, :])

            def proj_nc(wdram, bias_t, ncx, out_cb, split_w=1):
                """y[n 128, s 1024] = dequant(w_chunk @ x) for n-chunk ncx."""
                wt = wp.tile([128, KC, 128], BF16, tag="w")
                kq = KC // split_w
                for j in range(split_w):
                    nc.scalar.dma_start(wt[:, j * kq:(j + 1) * kq, :],
                                        wdram[ncx, :, j * kq * 128:
                                              (j + 1) * kq * 128])
                y = pscr.tile([128, SC], F32, tag="y")
                for half in range(ST):
                    fsl = bass.ts(half, 512)
                    ps = pps.tile([128, 512], F32, tag="ps")
                    for kc in range(KC):
                        nc.tensor.matmul(ps[:], wt[:, kc, :],
                                         xts[kc][:, fsl],
                                         start=(kc == 0), stop=(kc == KC - 1))
                    nc.scalar.activation(y[:, fsl], ps[:], AF.Identity,
                                         bias=bias_t[:, ncx:ncx + 1],
                                         scale=ALPHA)
                out_cb(ncx, y)

            def q_like(dst, ncx, y):
                """quant -> RoPE (fp32) -> quant -> bf16 [d 128, s 1024].

                y is clobbered (quant pass1 is in-place) and then reused as
                the rotate-half product buffer; qi is rotated in place."""
                qi = pscr.tile([128, SC], F32, tag="qi")
                _quant2(nc, qi[:], y[:])
                nc.vector.tensor_mul(y[0:64, :], qi[64:128, :],
                                     sinrT[64:128, :])
                nc.vector.tensor_mul(y[64:128, :], qi[0:64, :],
                                     sinrT[0:64, :])
                nc.vector.tensor_mul(qi[:], qi[:], cosT[:])
                nc.vector.tensor_add(qi[:], qi[:], y[:])
                _quant2(nc, dst[:], qi[:])

            def v_like(dst_vn, ncx, y):
                vt = pscr.tile([128, SC], BF16, tag="vt")
                _quant2(nc, vt[:], y[:])
                # v^T [d, t] -> v natural [t, d] via DMA transpose (2-byte)
                nc.sync.dma_start_transpose(dst_vn[:], vt[:])

            def qk_head(qTh, kTh, ptr):
                """Single-pass softmax, normal layout; fills ptr [t,s] bf16."""
                for sc in range(NC_):
                    pa = aps.tile([128, SC], F32, tag="attn")
                    for half in range(ST):
                        fsl = bass.ts(half, 512)
                        nc.tensor.matmul(pa[:, fsl], qTh[:, bass.ts(sc, 128)],
                                         kTh[:, fsl], start=True, stop=True)
                    mx = stat.tile([128, 1], F32, tag="mx")
                    nc.vector.reduce_max(mx[:], pa[:], axis=AX.X)
                    nb = stat.tile([128, 1], F32, tag="nb")
                    nc.vector.tensor_scalar_mul(nb[:], mx[:], -CATT)
                    sm = stat.tile([128, 1], F32, tag="sm")
                    et = etp.tile([128, SC], F32, tag="et")
                    nc.scalar.activation(et[:], pa[:], AF.Exp,
                                         bias=nb[:], scale=CATT,
                                         accum_out=sm[:])
                    smk = stat.tile([128, 1], F32, tag="smk")
                    nc.vector.tensor_scalar_mul(smk[:], sm[:], 1.0 / 127.0)
                    rs = stat.tile([128, 1], F32, tag="rs")
                    nc.vector.reciprocal(rs[:], smk[:])
                    # p~ = et * (127/sum), magic-rounded to ints, bf16
                    # (first pass in-place on et)
                    nc.vector.tensor_scalar(et[:], et[:], rs[:], MAGIC,
                                            OP.mult, OP.add)
                    pb = pbp.tile([128, SC], BF16, tag="pb")
                    last = nc.vector.tensor_scalar_add(pb[:], et[:], -MAGIC)
                    # [s-chunk 128, t 1024] -> ptr[t%128, sc, t//128, 128]
                    nc.sync.dma_start_transpose(ptr[:, sc, :, :], pb[:])
                return last

            def pv_head(h, vn, ptr):
                for half in range(ST):
                    fsl = bass.ts(half, 512)
                    pc = cps.tile([128, 512], F32, tag="ctx")
                    for c in range(NC_):
                        nc.tensor.matmul(pc[:], vn[:, c, :],
                                         ptr[:, half * 4:half * 4 + 4, c, :],
                                         start=(c == 0), stop=(c == NC_ - 1))
                    cf = pscr.tile([128, 512], F32, tag="cf")
                    nc.scalar.activation(cf[:], pc[:], AF.Copy, scale=C3)
                    _quant2(nc, ctxT[:, h, fsl], cf[:])

            # ---- head-pipelined projections + attention ----
            # pv lags one head behind qk so the PE has projection work to
            # chew on while each head's softmax chain completes.
            prev = None
            for h in range(HPC):
                qTh = qkp.tile([128, SC], BF16, tag="qT")
                kTh = qkp.tile([128, SC], BF16, tag="kT")
                vn = vnp.tile([128, NC_, 128], BF16, tag="vn")
                ptr = ptrp.tile([128, NC_, NC_, 128], BF16, tag="ptr")
                sw = 4 if h == 0 else 1
                proj_nc(d["wq"], bq, h, lambda i, y: q_like(qTh, i, y), sw)
                proj_nc(d["wk"], bk, h, lambda i, y: q_like(kTh, i, y), sw)
                proj_nc(d["wv"], bv, h, lambda i, y: v_like(vn, i, y), sw)
                qk_head(qTh, kTh, ptr)
                if prev is not None:
                    pv_head(*prev)
                prev = (h, vn, ptr)
            pv_head(*prev)

        # ---------------- o_proj partial ----------------
        with tc.tile_pool(name="wo", bufs=3) as wop, \
             tc.tile_pool(name="ops", bufs=4, space="PSUM") as ops, \
             tc.tile_pool(name="oscr", bufs=3) as oscr:
            for mc in range(32):
                wt = wop.tile([128, NC_, 128], BF16, tag="wo")
                nc.scalar.dma_start(wt[:], d["wo"][mc, :, :])
                for half in range(ST):
                    fsl = bass.ts(half, 512)
                    po = ops.tile([128, 512], F32, tag="po")
                    for kcx in range(NC_):
                        nc.tensor.matmul(po[:], wt[:, kcx, :],
                                         ctxT[:, kcx, fsl],
                                         start=(kcx == 0), stop=(kcx == NC_ - 1))
                    ob = oscr.tile([128, 512], F32, tag="ob")
                    nc.scalar.copy(ob[:], po[:])
                    nc.sync.dma_start(d["ot"][bass.ts(mc, 128), fsl], ob[:])


# ---------------- host side ----------------

def _rope_tables_np(pos_row):
    j = np.arange(0, HD, 2, dtype=np.float32) / np.float32(HD)
    inv = np.float32(1.0) / np.power(np.float32(THETA), j)
    freqs = pos_row.astype(np.float32)[:, None] * inv[None, :]   # [S, 64]
    emb = np.concatenate([freqs, freqs], axis=-1)                # [S, 128]
    cosT = np.ascontiguousarray(np.cos(emb).T.astype(np.float32))
    sinT = np.sin(emb).T.astype(np.float32)
    sinr = sinT.copy()
    sinr[0:HD // 2] *= np.float32(-1.0)
    # rotate by 64 partitions so rope muls have base-aligned inputs:
    # sins[d] = sinrot[(d+64) % 128]
    sins = np.concatenate([sinr[HD // 2:], sinr[:HD // 2]], axis=0)
    return cosT, np.ascontiguousarray(sins)


def _prep_inputs(hidden_states, position_ids, w_q, w_k, w_v, w_o,
                 b_q, b_k, b_v):
    bf = ml_dtypes.bfloat16
    in_maps = []
    x = np.asarray(hidden_states, dtype=np.float32)
    x_i8 = np.clip(np.round(x / np.float32(S_IN)), -128, 127)
    for c in range(NCORES):
        b, g = c // TPG, c % TPG
        gsl = slice(g * NSL, (g + 1) * NSL)
        xt = np.ascontiguousarray(x_i8[b].T).reshape(KC, 128, SC).astype(bf)
        def wslice(w):
            wg = np.asarray(w[gsl], dtype=np.float32)     # [1024, 4096]
            t = wg.reshape(NC_, 128, KC, 128).transpose(0, 3, 2, 1)
            return np.ascontiguousarray(t.reshape(NC_, 128, HID)).astype(bf)
        wog = np.asarray(w_o[:, gsl], dtype=np.float32)   # [4096, 1024]
        wo = wog.reshape(32, 128, NC_, 128).transpose(0, 3, 2, 1)
        wo = np.ascontiguousarray(wo.reshape(32, 128, NSL)).astype(bf)
        cosT, sinrT = _rope_tables_np(np.asarray(position_ids)[b])
        bs = lambda bb, s: np.ascontiguousarray(
            (np.asarray(bb[gsl], dtype=np.float32) * np.float32(s))
            .reshape(NC_, 128).T)
        in_maps.append({
            "xt": xt, "wq": wslice(w_q), "wk": wslice(w_k), "wv": wslice(w_v),
            "wo": wo, "cosT": cosT, "sinrT": sinrT,
            "bq": bs(b_q, S_B / S_Q), "bk": bs(b_k, S_B / S_K),
            "bv": bs(b_v, S_B / S_V),
        })
    return in_maps


def _finish(results, b_o):
    out = np.empty((B, S, HID), dtype=np.float32)
    sc = np.float32(S_O * S_W)
    bo = np.asarray(b_o, dtype=np.float32)
    for b in range(B):
        acc = np.zeros((HID, SC), dtype=np.float32)
        for g in range(TPG):
            acc += results[b * TPG + g]["ot"]
        out[b] = acc.T * sc + bo[None, :]
    return out


def kernel(hidden_states, position_ids, w_q, w_k, w_v, w_o,
           b_q, b_k, b_v, b_o):
    if "nc" not in _CACHE:
        _CACHE["nc"] = build_nc()
    nc = _CACHE["nc"]
    in_maps = _prep_inputs(hidden_states, position_ids, w_q, w_k, w_v, w_o,
                           b_q, b_k, b_v)
    res = run_bass_kernel_spmd(nc, in_maps, core_ids=list(range(NCORES)))
    return _finish(res.results, b_o)
